# revision 1
# baseline (speedup 1.0000x reference)
"""Trainium2 Bass kernel for nn_CompositionBlock (gnn_message_passing).

Data-parallel over batch B=8 across 8 NeuronCores (one sample per core).
Transposed layout: components (p/o) on partitions of the big bilinear
intermediates; TensorE does the contractions AND both reduction passes (via
tiny selection-matrix matmuls); VectorE only does the per-token elementwise
multiply, reading PSUM directly; biases ride ScalarE's per-partition bias
port; the head-scatter is a one-hot matmul.
"""

import copy
import json

import numpy as np

B, S, T, D, P = 8, 256, 128, 64, 128
NCORES = 8
JT = S // 128  # token tiles per core


# ----------------------------------------------------------------------------
# Compat: the walrus build in this container accepts at most one sync-wait on
# CTRL-class instructions, but TileContext's tail drain packs several. Split
# any multi-wait instruction into a chain of single-wait clones.
# ----------------------------------------------------------------------------
def _split_multiwait_bir(bir_json_bytes: bytes) -> bytes:
    bir = json.loads(bir_json_bytes)
    for func in bir.get("functions", []):
        for bb in func.get("blocks", []):
            new_instructions = []
            for ins in bb.get("instructions", []):
                si = ins.get("sync_info") or {}
                waits = si.get("on_wait") or []
                if len(waits) > 1:
                    # hoist all but the last wait onto same-engine NoOps,
                    # executed in order by the engine's sequencer just before
                    # the original instruction
                    for i, w in enumerate(waits[:-1]):
                        new_instructions.append({
                            "debug": ins.get("debug", 0),
                            "engine": ins["engine"],
                            "ins": [],
                            "name": f"{ins['name']}_w{i}",
                            "opcode": "NoOp",
                            "outs": [],
                            "sync_info": {"on_wait": [w], "on_update": []},
                        })
                    ins["sync_info"] = {
                        "on_wait": [waits[-1]],
                        "on_update": si.get("on_update") or [],
                    }
                new_instructions.append(ins)
            bb["instructions"] = new_instructions
    return json.dumps(bir).encode()


def _install_compat():
    import concourse.bass_utils as bu

    if getattr(bu.compile_bir_kernel, "_multiwait_patched", False):
        return
    orig = bu.compile_bir_kernel

    def patched(bir_json, tmpdir, neff_name="file.neff"):
        return orig(_split_multiwait_bir(bir_json), tmpdir, neff_name)

    patched._multiwait_patched = True
    bu.compile_bir_kernel = patched
    try:
        import concourse.bass2jax as b2j

        if getattr(b2j, "compile_bir_kernel", None) is not None:
            b2j.compile_bir_kernel = patched
    except ImportError:
        pass


B, S, T, D, P = 8, 256, 128, 64, 128
NCORES = 8
JT = S // 128

_NC_CACHE = {}


def build_nc():
    if "nc" in _NC_CACHE:
        return _NC_CACHE["nc"]
    import concourse.bass as bass
    import concourse.tile as tile
    from concourse import mybir
    from concourse.masks import make_identity

    f32 = mybir.dt.float32
    f16 = mybir.dt.float16
    Alu = mybir.AluOpType
    Act = mybir.ActivationFunctionType

    nc = bass.Bass(trn_type="TRN2")

    tokT_d = nc.dram_tensor("tokT", [T, S], f16, kind="ExternalInput")
    tokT8_d = nc.dram_tensor("tokT8", [8, 128, 512], f32, kind="ExternalInput")
    depT8_d = nc.dram_tensor("depT8", [4, 128, 512], f32, kind="ExternalInput")
    w1t_d = nc.dram_tensor("w1t", [T, 64 * 128], f16, kind="ExternalInput")
    w2t_d = nc.dram_tensor("w2t", [P, 128 * 128], f16, kind="ExternalInput")
    red_d = nc.dram_tensor("red", [128, 64], f16, kind="ExternalInput")
    headsf_d = nc.dram_tensor("headsf", [JT, 128, 1], f32, kind="ExternalInput")
    wr_d = nc.dram_tensor("wr", [JT, 128, 1], f32, kind="ExternalInput")
    iota_d = nc.dram_tensor("iota", [1, S], f32, kind="ExternalInput")
    bdep_d = nc.dram_tensor("bdep", [128, 1], f32, kind="ExternalInput")
    bcomp_d = nc.dram_tensor("bcomp", [128, 1], f32, kind="ExternalInput")
    base_d = nc.dram_tensor("base", [128, 1], f32, kind="ExternalInput")
    c0_d = nc.dram_tensor("c0", [1, T], f32, kind="ExternalInput")
    out_d = nc.dram_tensor("out", [S, T], f32, kind="ExternalOutput")

    def bcast_row(dram_ap):
        return bass.AP(
            tensor=dram_ap.tensor,
            offset=dram_ap.offset,
            ap=[[0, 128]] + list(dram_ap.ap[1:]),
        )

    with tile.TileContext(nc) as tc:
        with (
            tc.tile_pool(name="consts", bufs=1) as consts,
            tc.tile_pool(name="weights", bufs=1) as weights,
            tc.tile_pool(name="work", bufs=4) as work,
            tc.tile_pool(name="keep", bufs=1) as keep,
            tc.tile_pool(name="psmm", bufs=3, space="PSUM") as psmm,
            tc.tile_pool(name="pstde", bufs=1, space="PSUM") as pstde,
            tc.tile_pool(name="pscomp", bufs=1, space="PSUM") as pscomp,
            tc.tile_pool(name="psfin", bufs=1, space="PSUM") as psfin,
        ):
            ident16 = consts.tile([128, 128], f16)
            make_identity(nc, ident16)

            iota_b = consts.tile([128, S], f32)
            nc.sync.dma_start(out=iota_b, in_=bcast_row(iota_d[:, :]))
            c0_b = consts.tile([128, T], f32)
            nc.sync.dma_start(out=c0_b, in_=bcast_row(c0_d[:, :]))
            bdep_c = consts.tile([128, 1], f32)
            nc.sync.dma_start(out=bdep_c, in_=bdep_d[:, :])
            bcomp_c = consts.tile([128, 1], f32)
            nc.sync.dma_start(out=bcomp_c, in_=bcomp_d[:, :])
            base_c = consts.tile([128, 1], f32)
            nc.sync.dma_start(out=base_c, in_=base_d[:, :])
            red_flat = consts.tile([128, 64], f16)
            nc.sync.dma_start(out=red_flat, in_=red_d[:, :])
            red_sb = red_flat.rearrange("q (p m) -> q p m", m=32)

            tokT_sb = consts.tile([128, S], f16)
            nc.sync.dma_start(out=tokT_sb, in_=tokT_d[:, :])

            # replicated per-token multiplier tiles (host-precomputed)
            tokT8_sb = []
            for i in range(8):
                t = weights.tile([128, 512], f32, name=f"tokT8_{i}", tag=f"tokT8_{i}")
                nc.gpsimd.dma_start(out=t, in_=tokT8_d[i, :, :])
                tokT8_sb.append(t)
            depT8_sb = []
            for i in range(4):
                t = weights.tile([128, 512], f32, name=f"depT8_{i}", tag=f"depT8_{i}")
                nc.gpsimd.dma_start(out=t, in_=depT8_d[i, :, :])
                depT8_sb.append(t)

            # weight chunk tiles: w1t packed 8 chunks/tile, w2t 8 chunks/tile
            w1t_sb = []
            for i in range(8):
                t = weights.tile([128, 8 * 128], f16, name=f"w1t_{i}", tag=f"w1t_{i}")
                eng = [nc.sync, nc.scalar][i % 2]
                eng.dma_start(out=t, in_=w1t_d[:, i * 1024 : (i + 1) * 1024])
                w1t_sb.append(t)
            w2t_sb = []
            for i in range(16):
                t = weights.tile([128, 8 * 128], f16, name=f"w2t_{i}", tag=f"w2t_{i}")
                eng = [nc.sync, nc.scalar][i % 2]
                eng.dma_start(out=t, in_=w2t_d[:, i * 1024 : (i + 1) * 1024])
                w2t_sb.append(t)

            headsf_t = []
            wr_t = []
            for jt in range(JT):
                ht = consts.tile([128, 1], f32, name=f"hf{jt}", tag=f"hf{jt}")
                nc.sync.dma_start(out=ht, in_=headsf_d[jt, :, :])
                headsf_t.append(ht)
                wt = consts.tile([128, 1], f32, name=f"wr{jt}", tag=f"wr{jt}")
                nc.sync.dma_start(out=wt, in_=wr_d[jt, :, :])
                wr_t.append(wt)

            # ---- stage 1 ----
            tde_ps = pstde.tile([128, S], f32)
            for cp in range(32):  # chunk pairs (c=2cp, 2cp+1)
                c0i, c1i = 2 * cp, 2 * cp + 1
                ps = psmm.tile([128, 512], f32, name="mm", tag="mm")
                for k, c in enumerate((c0i, c1i)):
                    nc.tensor.matmul(
                        ps[:, k * 256 : (k + 1) * 256],
                        w1t_sb[c // 8][:, (c % 8) * 128 : (c % 8 + 1) * 128],
                        tokT_sb,
                    )
                prod = work.tile([128, 512], f16, name="prod1", tag="prod1")
                b = c0i % 8  # b, b+1 pair
                nc.vector.tensor_tensor(
                    out=prod, in0=ps, in1=depT8_sb[b // 2], op=Alu.mult
                )
                for k, c in enumerate((c0i, c1i)):
                    a = c // 8
                    g = a // 2
                    nc.tensor.matmul(
                        tde_ps[32 * g : 32 * g + 32, :],
                        red_sb[:, a % 2, :],
                        prod[:, k * 256 : (k + 1) * 256],
                        start=(c % 16 == 0),
                        stop=(c % 16 == 15),
                        tile_position=(0, 32 * g),
                    )
            hT = keep.tile([128, S], f16)
            nc.scalar.activation(hT, tde_ps, Act.Tanh, bias=bdep_c)

            # ---- stage 2 ----
            comp_ps = pscomp.tile([128, S], f32)
            for cp in range(64):
                c0i, c1i = 2 * cp, 2 * cp + 1
                ps2 = psmm.tile([128, 512], f32, name="mm", tag="mm")
                for k, c in enumerate((c0i, c1i)):
                    nc.tensor.matmul(
                        ps2[:, k * 256 : (k + 1) * 256],
                        w2t_sb[c // 8][:, (c % 8) * 128 : (c % 8 + 1) * 128],
                        hT,
                    )
                prod2 = work.tile([128, 512], f16, name="prod2", tag="prod2")
                b = c0i % 16
                nc.vector.tensor_tensor(
                    out=prod2, in0=ps2, in1=tokT8_sb[b // 2], op=Alu.mult
                )
                for k, c in enumerate((c0i, c1i)):
                    a = c // 16
                    g = a // 2
                    nc.tensor.matmul(
                        comp_ps[32 * g : 32 * g + 32, :],
                        red_sb[:, a % 2, :],
                        prod2[:, k * 256 : (k + 1) * 256],
                        start=(c % 32 == 0),
                        stop=(c % 32 == 31),
                        tile_position=(0, 32 * g),
                    )

            specT = work.tile([128, S], f32, name="specT", tag="specT")
            nc.scalar.activation(specT, comp_ps, Act.Tanh, bias=bcomp_c)
            deltaT = keep.tile([128, S], f16)
            nc.vector.tensor_scalar(
                out=deltaT, in0=specT, scalar1=base_c, scalar2=None,
                op0=Alu.subtract,
            )

            # transpose deltaT -> delta[j, o] per token tile, build soh, final
            delta_sb = []
            for jt in range(JT):
                dps = psmm.tile([128, 512], f16, name="mm", tag="mm")
                nc.tensor.transpose(
                    dps[:, 0:128], deltaT[:, jt * 128 : (jt + 1) * 128], ident16
                )
                dsb = keep.tile([128, 128], f16, name=f"delta{jt}", tag=f"delta{jt}")
                nc.scalar.copy(dsb, dps[:, 0:128])
                delta_sb.append(dsb)

            soh = []
            for jt in range(JT):
                s = keep.tile([128, S], f16, name=f"soh{jt}", tag=f"soh{jt}")
                nc.vector.tensor_scalar(
                    out=s, in0=iota_b, scalar1=headsf_t[jt], scalar2=wr_t[jt],
                    op0=Alu.is_equal, op1=Alu.mult,
                )
                soh.append(s)

            fin_ps = psfin.tile([128, S], f32)
            for ic in range(2):
                for jt in range(JT):
                    nc.tensor.matmul(
                        fin_ps[:, ic * 128 : (ic + 1) * 128],
                        soh[jt][:, ic * 128 : (ic + 1) * 128],
                        delta_sb[jt],
                        start=(jt == 0),
                        stop=(jt == JT - 1),
                    )
            for ic in range(2):
                outsb = work.tile([128, T], f32, name="outsb", tag="outsb")
                nc.vector.tensor_add(
                    outsb, fin_ps[:, ic * 128 : (ic + 1) * 128], c0_b
                )
                nc.sync.dma_start(
                    out=out_d[ic * 128 : (ic + 1) * 128, :], in_=outsb
                )

    _NC_CACHE["nc"] = nc
    return nc


def prep_core_inputs(token_embeddings, dep_embeddings, dep_heads,
                     W_dep, b_dep, W_comp, b_comp, W_red, b_red):
    f32 = np.float32
    f16 = np.float16
    tok = np.asarray(token_embeddings, dtype=f32)
    dep = np.asarray(dep_embeddings, dtype=f32)
    heads = np.asarray(dep_heads)
    W_dep = np.asarray(W_dep, dtype=f32)
    b_dep = np.asarray(b_dep, dtype=f32)
    W_comp = np.asarray(W_comp, dtype=f32)
    b_comp = np.asarray(b_comp, dtype=f32)
    wr = np.asarray(W_red, dtype=f32)[0]
    b_red = np.asarray(b_red, dtype=f32)

    # w1t[(a,b), t, (p'*8+d')] = W_dep[16a+p', t, 8b+d']
    X = W_dep.reshape(8, 16, T, 8, 8)            # [a, p', t, b, d']
    w1t = np.ascontiguousarray(
        X.transpose(2, 0, 3, 1, 4).reshape(T, 64 * 128)
    ).astype(f16)                                # [t, ((a,b), (p',d'))]
    # w2t[(a,b), p, (o'*8+t')] = W_comp[16a+o', 8b+t', p]
    Y = W_comp.reshape(8, 16, 16, 8, P)          # [a, o', b, t', p]
    w2t = np.ascontiguousarray(
        Y.transpose(4, 0, 2, 1, 3).reshape(P, 128 * 128)
    ).astype(f16)                                # [p, ((a,b), (o',t'))]
    # red[par][r, m] = (m == 16*par + r//8)
    r = np.arange(128)
    red = np.zeros((128, 2, 32), dtype=f16)
    for par in range(2):
        red[r, par, 16 * par + r // 8] = 1.0
    red = red.reshape(128, 64)

    base = np.tanh(b_comp)
    c0 = (base * wr.sum() + b_red[0]).astype(f32)
    iota = np.arange(S, dtype=f32).reshape(1, S)
    headsf = heads.astype(f32).reshape(B, JT, 128, 1)
    wr_t = np.ascontiguousarray(wr.reshape(JT, 128, 1))

    shared = {
        "w1t": w1t, "w2t": w2t, "red": red,
        "iota": iota,
        "bdep": b_dep.reshape(128, 1),
        "bcomp": b_comp.reshape(128, 1),
        "base": base.reshape(128, 1).astype(f32),
        "c0": c0.reshape(1, T),
    }
    in_maps = []
    for c in range(NCORES):
        tokc = tok[c]                             # [S, T]
        depc = dep[c]                             # [S, D]
        tokTc = np.ascontiguousarray(tokc.T)      # [T, S]
        tokT8 = np.empty((16, 128, S), dtype=f32)
        for b in range(16):
            tokT8[b] = np.tile(tokTc[8 * b : 8 * b + 8, :], (16, 1))
        depT = depc.T                             # [D, S]
        depT8 = np.empty((8, 128, S), dtype=f32)
        for b in range(8):
            depT8[b] = np.tile(depT[8 * b : 8 * b + 8, :], (16, 1))
        m = dict(shared)
        m["tokT"] = tokTc.astype(f16)
        m["tokT8"] = np.ascontiguousarray(
            tokT8.reshape(8, 2, 128, S).transpose(0, 2, 1, 3).reshape(8, 128, 2 * S)
        )
        m["depT8"] = np.ascontiguousarray(
            depT8.reshape(4, 2, 128, S).transpose(0, 2, 1, 3).reshape(4, 128, 2 * S)
        )
        m["headsf"] = np.ascontiguousarray(headsf[c])
        m["wr"] = wr_t
        in_maps.append(m)
    return in_maps


def kernel(**inputs) -> np.ndarray:
    _install_compat()
    from concourse.bass_utils import run_bass_kernel_spmd

    nc = build_nc()
    in_maps = prep_core_inputs(**inputs)
    res = run_bass_kernel_spmd(nc, in_maps, core_ids=list(range(NCORES)))
    out = np.stack([res.results[c]["out"] for c in range(NCORES)], axis=0)
    return out.astype(np.float32)


# aliases used by test harness
_build_nc = build_nc
_prep_core_inputs = prep_core_inputs

